# revision 10
# baseline (speedup 1.0000x reference)
"""Trainium2 Bass kernel for per-head attention.

Problem shapes: x [4, 1024, 12, 768]; per-head weights W_Q/K/V [12, 768, 64],
W_O [12, 64, 768]; the output projection keeps the head axis, so each of the
48 (batch, head) pairs is fully independent. Sharding: 6 pairs per core
across 8 NeuronCores (SPMD — same program, different per-core inputs).

Per-pair device pipeline (x_bh [S=1024, DM=768], S-tiles of 128):
  - host supplies xT [DM, S]; qT/kT/vT [64, S] come from matmuls with the
    128-row weight chunks as stationary operands (fp32r = TF32 rate).
  - scores are computed transposed (scoresT[k, q], k on partitions), causally
    chunked, so only the lower triangle is ever computed; softmax skips the
    max-subtraction (|scores| <~ 3; masked lanes use exp underflow semantics
    via a 0/1 mask multiply on the diagonal blocks).
  - a ones-column appended to v (v_aug [128, 65], built by PE-transposing
    vT tiles) makes the z-matmul also produce the softmax denominator
    (row 64 of zT_aug).
  - output projection uses Wo_aug whose row 64 is b_V @ W_O + b_O/H; dividing
    the projected result by the denominator (per-partition scalar, obtained
    by PE-transposing zT_aug s-tile slices) yields exactly
    softmax(scores) @ v @ W_O + b_V @ W_O + b_O/H.
"""

import numpy as np

import concourse.bacc as bacc
import concourse.mybir as mybir
from concourse.bass_utils import run_bass_kernel_spmd
from concourse.tile import TileContext

F32 = mybir.dt.float32
F32R = mybir.dt.float32r

B, S, H, DM, DH = 4, 1024, 12, 768, 64
N_CORES = 8
PAIRS_PER_CORE = (B * H) // N_CORES  # 6
MC = DM // 128  # m-chunks
ST = S // 128   # s-tiles
QC = S // 512   # q-chunks


def _build_kernel(n_pairs=PAIRS_PER_CORE):
    nc = bacc.Bacc()

    xT = nc.declare_dram_parameter("xT", [n_pairs, DM, S], F32R, isOutput=False)
    # packed [Wk | Wq] stationary chunks: kT lands on psum rows 0:64 (aligned
    # for the scores lhsT), qT on rows 64:128 (partition-shifted to base 0 by
    # an SBUF->SBUF DMA afterwards)
    wqk = nc.declare_dram_parameter("wqk", [n_pairs, MC, 128, 128], F32R, isOutput=False)
    wv = nc.declare_dram_parameter("wv", [n_pairs, MC, 128, DH], F32R, isOutput=False)
    wo = nc.declare_dram_parameter("wo", [n_pairs, DH + 1, DM], F32R, isOutput=False)
    bq = nc.declare_dram_parameter("bq", [n_pairs, 128, 1], F32, isOutput=False)
    bk = nc.declare_dram_parameter("bk", [n_pairs, DH, 1], F32, isOutput=False)
    ident = nc.declare_dram_parameter("ident", [128, 128], F32, isOutput=False)
    maskT = nc.declare_dram_parameter("maskT", [128, 128], F32R, isOutput=False)
    out = nc.declare_dram_parameter("out", [n_pairs, S, DM], F32, isOutput=True)

    with TileContext(nc) as tc:
        with (
            tc.tile_pool(name="const", bufs=1) as pconst,
            tc.tile_pool(name="xt", bufs=3) as px,
            tc.tile_pool(name="w", bufs=3) as pw,
            tc.tile_pool(name="qkv", bufs=2) as pqkv,
            tc.tile_pool(name="vaug", bufs=2) as pva,
            tc.tile_pool(name="exp", bufs=3) as pexp,
            tc.tile_pool(name="z", bufs=2) as pz,
            tc.tile_pool(name="rc", bufs=2) as prc,
            tc.tile_pool(name="outb", bufs=3) as pout,
            tc.tile_pool(name="ps_qkv", bufs=2, space="PSUM") as ppq,
            tc.tile_pool(name="ps_s", bufs=3, space="PSUM") as pps,
            tc.tile_pool(name="ps_z", bufs=1, space="PSUM") as ppz,
            # transposes + output-projection psums share two bank slots
            tc.tile_pool(name="ps_mix", bufs=2, space="PSUM") as ppmix,
        ):
            ident_t = pconst.tile([128, 128], F32, name="ident_t")
            nc.sync.dma_start(out=ident_t[:], in_=ident[:])
            mask_t = pconst.tile([128, 128], F32R, name="mask_t")
            nc.sync.dma_start(out=mask_t[:], in_=maskT[:])

            for p in range(n_pairs):
                xt = []
                for mc in range(MC):
                    t = px.tile([128, S], F32R, name=f"xt{mc}", tag=f"xt{mc}")
                    nc.sync.dma_start(out=t[:], in_=xT[p, mc * 128:(mc + 1) * 128, :])
                    xt.append(t)
                wqk_t = pw.tile([128, MC, 128], F32R, name="wqk_t", tag="wqk")
                nc.sync.dma_start(out=wqk_t[:], in_=wqk[p].rearrange("c p d -> p c d"))
                wv_t = pw.tile([128, MC, DH], F32R, name="wv_t", tag="wv")
                nc.sync.dma_start(out=wv_t[:], in_=wv[p].rearrange("c p d -> p c d"))
                wo_t = pw.tile([DH + 1, DM], F32R, name="wo_t", tag="wo")
                nc.sync.dma_start(out=wo_t[:], in_=wo[p])
                bq_t = pw.tile([128, 1], F32, name="bq_t", tag="bq")
                nc.sync.dma_start(out=bq_t[:], in_=bq[p])
                bk_t = pw.tile([DH, 1], F32, name="bk_t", tag="bk")
                nc.sync.dma_start(out=bk_t[:], in_=bk[p])

                # QK projection, packed M=128 stationary -> [kT; qT] psum
                qT_sb = pqkv.tile([DH, S], F32R, name="qT_sb", tag="qT")
                kT_sb = pqkv.tile([DH, S], F32R, name="kT_sb", tag="kT")
                qstage = pqkv.tile([128, S], F32R, name="qstage", tag="qst")
                vT_sb = pqkv.tile([DH + 1, S], F32, name="vT_sb", tag="vT")
                nc.vector.memset(vT_sb[DH:DH + 1, :], 1.0)

                for sc in range(QC):
                    ps = ppq.tile([128, 512], F32, name="ps_qk", tag="ps_qkv")
                    for mc in range(MC):
                        nc.tensor.matmul(
                            ps[:], wqk_t[:, mc, :],
                            xt[mc][:, sc * 512:(sc + 1) * 512],
                            start=(mc == 0), stop=(mc == MC - 1))
                    cols = slice(sc * 512, (sc + 1) * 512)
                    nc.scalar.activation(
                        kT_sb[0:DH, cols], ps[0:DH, :],
                        mybir.ActivationFunctionType.Identity,
                        bias=bk_t[:], scale=1.0)
                    nc.scalar.activation(
                        qstage[DH:128, cols], ps[DH:128, :],
                        mybir.ActivationFunctionType.Identity,
                        bias=bq_t[DH:128, :], scale=1.0)
                    # partition shift 64:128 -> 0:64
                    nc.sync.dma_start(
                        out=qT_sb[0:DH, cols], in_=qstage[DH:128, cols])

                for sc in range(QC):
                    ps = ppq.tile([DH, 512], F32, name="ps_v", tag="ps_qkv")
                    for mc in range(MC):
                        nc.tensor.matmul(
                            ps[:], wv_t[:, mc, :],
                            xt[mc][:, sc * 512:(sc + 1) * 512],
                            start=(mc == 0), stop=(mc == MC - 1))
                    nc.vector.tensor_copy(
                        vT_sb[0:DH, sc * 512:(sc + 1) * 512], ps[:])

                # v_aug tiles [128, 65] via PE transpose
                v_aug = []
                for st in range(ST):
                    ps_t = ppmix.tile([128, DH + 1], F32, name="ps_vtr", tag="ps_mix")
                    nc.tensor.transpose(
                        ps_t[:], vT_sb[:, st * 128:(st + 1) * 128],
                        ident_t[0:DH + 1, 0:DH + 1])
                    va = pva.tile([128, DH + 1], F32R, name=f"va{st}", tag=f"va{st}")
                    nc.vector.tensor_copy(va[:], ps_t[:])
                    v_aug.append(va)

                # causal scoresT -> exp -> z accumulation.
                # The scores matmul runs two iterations ahead of the z matmul
                # so the PE never waits on the ACT exp in between.
                z_sb = pz.tile([DH + 1, S], F32R, name="z_sb", tag="z")
                for j in range(QC):
                    ps_z = ppz.tile([DH + 1, 512], F32, name="ps_z", tag="ps_z")
                    i_max = min(ST - 1, (512 * (j + 1) - 1) // 128)
                    pending = {}

                    def emit_scores(i, j=j):
                        c0 = max(128 * i, 512 * j)
                        L = 512 * (j + 1) - c0
                        ps_s = pps.tile([128, 512], F32, name="ps_s", tag="ps_s")
                        nc.tensor.matmul(
                            ps_s[:, 0:L], kT_sb[:, i * 128:(i + 1) * 128],
                            qT_sb[:, c0:c0 + L], start=True, stop=True)
                        pending[i] = (ps_s, c0, L)

                    emit_scores(0)
                    if i_max >= 1:
                        emit_scores(1)
                    for i in range(i_max + 1):
                        ps_s, c0, L = pending.pop(i)
                        ex = pexp.tile([128, 512], F32R, name="ex", tag="ex")
                        nc.scalar.activation(
                            ex[:, 0:L], ps_s[:, 0:L],
                            mybir.ActivationFunctionType.Exp,
                            bias=0.0, scale=0.125)
                        if c0 == 128 * i:
                            nc.vector.tensor_tensor(
                                ex[:, 0:128], ex[:, 0:128], mask_t[:],
                                op=mybir.AluOpType.mult)
                        if i + 2 <= i_max:
                            emit_scores(i + 2)
                        nc.tensor.matmul(
                            ps_z[:, c0 - 512 * j:512], v_aug[i][:], ex[:, 0:L],
                            start=(i == 0), stop=(i == i_max))
                    nc.vector.tensor_copy(z_sb[:, j * 512:(j + 1) * 512], ps_z[:])

                # denominators + output projection (normalize split DVE/ACT)
                for st in range(ST):
                    ps_t2 = ppmix.tile([128, DH + 1], F32, name="ps_ztr", tag="ps_mix")
                    nc.tensor.transpose(
                        ps_t2[:], z_sb[:, st * 128:(st + 1) * 128].bitcast(F32),
                        ident_t[0:DH + 1, 0:DH + 1])
                    rc = prc.tile([128, 1], F32, name=f"rc{st}", tag=f"rc{st}")
                    nc.vector.reciprocal(rc[:], ps_t2[:, DH:DH + 1])

                    ob = pout.tile([128, DM], F32, name="ob", tag="ob")
                    for o0, o1 in ((0, 512), (512, DM)):
                        ps_o = ppmix.tile([128, 512], F32, name="ps_o", tag="ps_mix")
                        nc.tensor.matmul(
                            ps_o[:, 0:o1 - o0], z_sb[:, st * 128:(st + 1) * 128],
                            wo_t[:, o0:o1], start=True, stop=True)
                        if o0 == 0:
                            nc.vector.tensor_scalar(
                                ob[:, o0:o1], ps_o[:, 0:o1 - o0], rc[:], None,
                                op0=mybir.AluOpType.mult)
                        else:
                            nc.scalar.mul(ob[:, o0:o1], ps_o[:, 0:o1 - o0], rc[:])
                    nc.sync.dma_start(
                        out=out[p, st * 128:(st + 1) * 128, :], in_=ob[:])

    nc.finalize()
    return nc


_NC_CACHE = {}


def _get_nc():
    if "nc" not in _NC_CACHE:
        _NC_CACHE["nc"] = _build_kernel()
    return _NC_CACHE["nc"]


def _make_pair_inputs(x, W_Q, b_Q, W_K, b_K, W_V, b_V, W_O, b_O, pairs):
    n = len(pairs)
    m = {
        "xT": np.empty((n, DM, S), np.float32),
        "wqk": np.empty((n, MC, 128, 128), np.float32),
        "wv": np.empty((n, MC, 128, DH), np.float32),
        "wo": np.empty((n, DH + 1, DM), np.float32),
        "bq": np.zeros((n, 128, 1), np.float32),
        "bk": np.empty((n, DH, 1), np.float32),
    }
    for idx, (b, h) in enumerate(pairs):
        m["xT"][idx] = x[b, :, h, :].T
        m["wqk"][idx, :, :, 0:DH] = W_K[h].reshape(MC, 128, DH)
        m["wqk"][idx, :, :, DH:128] = W_Q[h].reshape(MC, 128, DH)
        m["wv"][idx] = W_V[h].reshape(MC, 128, DH)
        m["wo"][idx, 0:DH] = W_O[h]
        m["wo"][idx, DH] = b_V[h] @ W_O[h] + b_O / H
        m["bq"][idx, DH:128, 0] = b_Q[h]
        m["bk"][idx] = b_K[h][:, None]
    m["ident"] = np.eye(128, dtype=np.float32)
    ql = np.arange(128)
    m["maskT"] = (ql[None, :] >= ql[:, None]).astype(np.float32)
    return m


def kernel(normalized_resid_pre, W_Q, b_Q, W_K, b_K, W_V, b_V, W_O, b_O):
    x = np.ascontiguousarray(np.asarray(normalized_resid_pre, dtype=np.float32))
    args = tuple(np.asarray(a, dtype=np.float32)
                 for a in (W_Q, b_Q, W_K, b_K, W_V, b_V, W_O, b_O))

    pairs = [(b, h) for b in range(B) for h in range(H)]
    nc = _get_nc()
    in_maps = [
        _make_pair_inputs(x, *args, pairs[c * PAIRS_PER_CORE:(c + 1) * PAIRS_PER_CORE])
        for c in range(N_CORES)
    ]
    res = run_bass_kernel_spmd(nc, in_maps, list(range(N_CORES)))

    got = np.empty((B, S, H, DM), np.float32)
    for c in range(N_CORES):
        for u in range(PAIRS_PER_CORE):
            b, h = pairs[c * PAIRS_PER_CORE + u]
            got[b, :, h, :] = res.results[c]["out"][u]
    return got


# revision 15
# speedup vs baseline: 1.1250x; 1.1250x over previous
"""Trainium2 Bass kernel for per-head attention.

Problem shapes: x [4, 1024, 12, 768]; per-head weights W_Q/K/V [12, 768, 64],
W_O [12, 64, 768]; the output projection keeps the head axis, so each of the
48 (batch, head) pairs is fully independent. Sharding: 6 pairs per core
across 8 NeuronCores (SPMD — same program, different per-core inputs).

Per-pair device pipeline (x_bh [S=1024, DM=768], S-tiles of 128):
  - host supplies xT [DM, S]; qT/kT/vT [64, S] come from matmuls with the
    128-row weight chunks as stationary operands (fp32r = TF32 rate).
  - scores are computed transposed (scoresT[k, q], k on partitions), causally
    chunked, so only the lower triangle is ever computed; softmax skips the
    max-subtraction (|scores| <~ 3; masked lanes use exp underflow semantics
    via a 0/1 mask multiply on the diagonal blocks).
  - a ones-column appended to v (v_aug [128, 65], built by PE-transposing
    vT tiles) makes the z-matmul also produce the softmax denominator
    (row 64 of zT_aug).
  - output projection uses Wo_aug whose row 64 is b_V @ W_O + b_O/H; dividing
    the projected result by the denominator (per-partition scalar, obtained
    by PE-transposing zT_aug s-tile slices) yields exactly
    softmax(scores) @ v @ W_O + b_V @ W_O + b_O/H.
"""

import numpy as np

import concourse.bacc as bacc
import concourse.mybir as mybir
from concourse.bass_utils import run_bass_kernel_spmd
from concourse.tile import TileContext

F32 = mybir.dt.float32
F32R = mybir.dt.float32r

B, S, H, DM, DH = 4, 1024, 12, 768, 64
N_CORES = 8
PAIRS_PER_CORE = (B * H) // N_CORES  # 6
MC = DM // 128  # m-chunks
ST = S // 128   # s-tiles
QC = S // 512   # q-chunks


def _build_kernel(n_pairs=PAIRS_PER_CORE):
    nc = bacc.Bacc()

    xT = nc.declare_dram_parameter("xT", [n_pairs, DM, S], F32R, isOutput=False)
    # one packed weight blob per pair (single DMA): columns are
    # [ wqk (MC*128, packed [Wk|Wq] chunks) | wv (MC*64) | wo_aug (768,
    #   rows 0:65) | bq (1) | bk (1) ]
    WQK0, WV0 = 0, MC * 128
    WO0 = WV0 + MC * DH
    BQ0 = WO0 + DM
    BK0 = BQ0 + 1
    WBL = BK0 + 1
    wb = nc.declare_dram_parameter("wb", [n_pairs, 128, WBL], F32R, isOutput=False)
    ident = nc.declare_dram_parameter("ident", [128, 128], F32, isOutput=False)
    maskT = nc.declare_dram_parameter("maskT", [128, 128], F32R, isOutput=False)
    out = nc.declare_dram_parameter("out", [n_pairs, S, DM], F32, isOutput=True)

    with TileContext(nc) as tc:
        with (
            tc.tile_pool(name="const", bufs=1) as pconst,
            tc.tile_pool(name="xt", bufs=2) as px,
            tc.tile_pool(name="w", bufs=3) as pw,
            tc.tile_pool(name="qkv", bufs=2) as pqkv,
            tc.tile_pool(name="vaug", bufs=2) as pva,
            tc.tile_pool(name="exp", bufs=3) as pexp,
            tc.tile_pool(name="z", bufs=2) as pz,
            tc.tile_pool(name="rc", bufs=2) as prc,
            tc.tile_pool(name="outb", bufs=3) as pout,
            tc.tile_pool(name="ps_qkv", bufs=2, space="PSUM") as ppq,
            tc.tile_pool(name="ps_s", bufs=3, space="PSUM") as pps,
            tc.tile_pool(name="ps_z", bufs=1, space="PSUM") as ppz,
            # transposes + output-projection psums share two bank slots
            tc.tile_pool(name="ps_mix", bufs=2, space="PSUM") as ppmix,
        ):
            ident_t = pconst.tile([128, 128], F32, name="ident_t")
            nc.sync.dma_start(out=ident_t[:], in_=ident[:])
            mask_t = pconst.tile([128, 128], F32R, name="mask_t")
            nc.sync.dma_start(out=mask_t[:], in_=maskT[:])

            for p in range(n_pairs):
                wb_t = pw.tile([128, WBL], F32R, name="wb_t", tag="wb")
                nc.sync.dma_start(out=wb_t[:], in_=wb[p])
                wqk_t = wb_t[:, WQK0:WV0].rearrange("p (c d) -> p c d", d=128)
                wv_t = wb_t[:, WV0:WO0].rearrange("p (c d) -> p c d", d=DH)
                wo_t = wb_t[0:DH + 1, WO0:WO0 + DM]
                bq_t = wb_t[:, BQ0:BQ0 + 1].bitcast(F32)
                bk_t = wb_t[0:DH, BK0:BK0 + 1].bitcast(F32)

                xta = px.tile([128, MC, S], F32R, name="xta", tag="xta")
                nc.sync.dma_start(
                    out=xta[:], in_=xT[p].rearrange("(c p) s -> p c s", p=128))
                xt = [xta[:, mc, :] for mc in range(MC)]

                # QK projection, packed M=128 stationary -> [kT; qT] psum
                qT_sb = pqkv.tile([DH, S], F32R, name="qT_sb", tag="qT")
                kT_sb = pqkv.tile([DH, S], F32R, name="kT_sb", tag="kT")
                qstage = pqkv.tile([128, S], F32R, name="qstage", tag="qst")
                vT_sb = pqkv.tile([DH + 1, S], F32, name="vT_sb", tag="vT")
                nc.vector.memset(vT_sb[DH:DH + 1, :], 1.0)

                for sc in range(QC):
                    ps = ppq.tile([128, 512], F32, name="ps_qk", tag="ps_qkv")
                    for mc in range(MC):
                        nc.tensor.matmul(
                            ps[:], wqk_t[:, mc, :],
                            xt[mc][:, sc * 512:(sc + 1) * 512],
                            start=(mc == 0), stop=(mc == MC - 1))
                    cols = slice(sc * 512, (sc + 1) * 512)
                    nc.scalar.activation(
                        kT_sb[0:DH, cols], ps[0:DH, :],
                        mybir.ActivationFunctionType.Identity,
                        bias=bk_t[:], scale=1.0)
                    nc.scalar.activation(
                        qstage[DH:128, cols], ps[DH:128, :],
                        mybir.ActivationFunctionType.Identity,
                        bias=bq_t[DH:128, :], scale=1.0)
                    # partition shift 64:128 -> 0:64
                    nc.sync.dma_start(
                        out=qT_sb[0:DH, cols], in_=qstage[DH:128, cols])

                for sc in range(QC):
                    ps = ppq.tile([DH, 512], F32, name="ps_v", tag="ps_qkv")
                    for mc in range(MC):
                        nc.tensor.matmul(
                            ps[:], wv_t[:, mc, :],
                            xt[mc][:, sc * 512:(sc + 1) * 512],
                            start=(mc == 0), stop=(mc == MC - 1))
                    nc.vector.tensor_copy(
                        vT_sb[0:DH, sc * 512:(sc + 1) * 512], ps[:])

                # v_aug tiles [128, 65] via PE transpose
                v_aug = []
                for st in range(ST):
                    ps_t = ppmix.tile([128, DH + 1], F32, name="ps_vtr", tag="ps_mix")
                    nc.tensor.transpose(
                        ps_t[:], vT_sb[:, st * 128:(st + 1) * 128],
                        ident_t[0:DH + 1, 0:DH + 1])
                    va = pva.tile([128, DH + 1], F32R, name=f"va{st}", tag=f"va{st}")
                    nc.vector.tensor_copy(va[:], ps_t[:])
                    v_aug.append(va)

                # causal scoresT -> exp -> z accumulation.
                # The scores matmul runs two iterations ahead of the z matmul
                # so the PE never waits on the ACT exp in between.
                z_sb = pz.tile([DH + 1, S], F32R, name="z_sb", tag="z")
                for j in range(QC):
                    ps_z = ppz.tile([DH + 1, 512], F32, name="ps_z", tag="ps_z")
                    i_max = min(ST - 1, (512 * (j + 1) - 1) // 128)
                    pending = {}

                    def emit_scores(i, j=j):
                        c0 = max(128 * i, 512 * j)
                        L = 512 * (j + 1) - c0
                        ps_s = pps.tile([128, 512], F32, name="ps_s", tag="ps_s")
                        nc.tensor.matmul(
                            ps_s[:, 0:L], kT_sb[:, i * 128:(i + 1) * 128],
                            qT_sb[:, c0:c0 + L], start=True, stop=True)
                        pending[i] = (ps_s, c0, L)

                    emit_scores(0)
                    if i_max >= 1:
                        emit_scores(1)
                    for i in range(i_max + 1):
                        ps_s, c0, L = pending.pop(i)
                        ex = pexp.tile([128, 512], F32R, name="ex", tag="ex")
                        nc.scalar.activation(
                            ex[:, 0:L], ps_s[:, 0:L],
                            mybir.ActivationFunctionType.Exp,
                            bias=0.0, scale=0.125)
                        if c0 == 128 * i:
                            nc.vector.tensor_tensor(
                                ex[:, 0:128], ex[:, 0:128], mask_t[:],
                                op=mybir.AluOpType.mult)
                        if i + 2 <= i_max:
                            emit_scores(i + 2)
                        nc.tensor.matmul(
                            ps_z[:, c0 - 512 * j:512], v_aug[i][:], ex[:, 0:L],
                            start=(i == 0), stop=(i == i_max))
                    nc.vector.tensor_copy(z_sb[:, j * 512:(j + 1) * 512], ps_z[:])

                # denominators + output projection (normalize split DVE/ACT);
                # outputs batched 4 s-tiles per DMA, issued on the idle
                # GpSimd (SWDGE) queue to offload the Sync sequencer
                obh = None
                for st in range(ST):
                    ps_t2 = ppmix.tile([128, DH + 1], F32, name="ps_ztr", tag="ps_mix")
                    nc.tensor.transpose(
                        ps_t2[:], z_sb[:, st * 128:(st + 1) * 128].bitcast(F32),
                        ident_t[0:DH + 1, 0:DH + 1])
                    rc = prc.tile([128, 1], F32, name=f"rc{st}", tag=f"rc{st}")
                    nc.vector.reciprocal(rc[:], ps_t2[:, DH:DH + 1])

                    g = st % 4
                    if g == 0:
                        obh = pout.tile([128, 4, DM], F32, name="obh", tag="obh")
                    for o0, o1 in ((0, 512), (512, DM)):
                        ps_o = ppmix.tile([128, 512], F32, name="ps_o", tag="ps_mix")
                        nc.tensor.matmul(
                            ps_o[:, 0:o1 - o0], z_sb[:, st * 128:(st + 1) * 128],
                            wo_t[:, o0:o1], start=True, stop=True)
                        if o0 == 0:
                            nc.vector.tensor_scalar(
                                obh[:, g, o0:o1], ps_o[:, 0:o1 - o0], rc[:], None,
                                op0=mybir.AluOpType.mult)
                        else:
                            nc.scalar.mul(obh[:, g, o0:o1], ps_o[:, 0:o1 - o0], rc[:])
                    if g == 3:
                        nc.gpsimd.dma_start(
                            out=out[p, (st - 3) * 128:(st + 1) * 128, :]
                            .rearrange("(g sp) m -> sp g m", sp=128),
                            in_=obh[:])

    nc.finalize()
    return nc


_NC_CACHE = {}


def _get_nc():
    if "nc" not in _NC_CACHE:
        _NC_CACHE["nc"] = _build_kernel()
    return _NC_CACHE["nc"]


def _make_pair_inputs(x, W_Q, b_Q, W_K, b_K, W_V, b_V, W_O, b_O, pairs):
    n = len(pairs)
    WQK0, WV0 = 0, MC * 128
    WO0 = WV0 + MC * DH
    BQ0 = WO0 + DM
    BK0 = BQ0 + 1
    WBL = BK0 + 1
    m = {
        "xT": np.empty((n, DM, S), np.float32),
        "wb": np.zeros((n, 128, WBL), np.float32),
    }
    for idx, (b, h) in enumerate(pairs):
        m["xT"][idx] = x[b, :, h, :].T
        wb = m["wb"][idx]
        wqk = wb[:, WQK0:WV0].reshape(128, MC, 128)
        wqk[:, :, 0:DH] = W_K[h].reshape(MC, 128, DH).transpose(1, 0, 2)
        wqk[:, :, DH:128] = W_Q[h].reshape(MC, 128, DH).transpose(1, 0, 2)
        wb[:, WV0:WO0].reshape(128, MC, DH)[:] = \
            W_V[h].reshape(MC, 128, DH).transpose(1, 0, 2)
        wb[0:DH, WO0:WO0 + DM] = W_O[h]
        wb[DH, WO0:WO0 + DM] = b_V[h] @ W_O[h] + b_O / H
        wb[DH:128, BQ0] = b_Q[h]
        wb[0:DH, BK0] = b_K[h]
    m["ident"] = np.eye(128, dtype=np.float32)
    ql = np.arange(128)
    m["maskT"] = (ql[None, :] >= ql[:, None]).astype(np.float32)
    return m


def kernel(normalized_resid_pre, W_Q, b_Q, W_K, b_K, W_V, b_V, W_O, b_O):
    x = np.ascontiguousarray(np.asarray(normalized_resid_pre, dtype=np.float32))
    args = tuple(np.asarray(a, dtype=np.float32)
                 for a in (W_Q, b_Q, W_K, b_K, W_V, b_V, W_O, b_O))

    pairs = [(b, h) for b in range(B) for h in range(H)]
    nc = _get_nc()
    in_maps = [
        _make_pair_inputs(x, *args, pairs[c * PAIRS_PER_CORE:(c + 1) * PAIRS_PER_CORE])
        for c in range(N_CORES)
    ]
    res = run_bass_kernel_spmd(nc, in_maps, list(range(N_CORES)))

    got = np.empty((B, S, H, DM), np.float32)
    for c in range(N_CORES):
        for u in range(PAIRS_PER_CORE):
            b, h = pairs[c * PAIRS_PER_CORE + u]
            got[b, :, h, :] = res.results[c]["out"][u]
    return got
